# revision 3
# baseline (speedup 1.0000x reference)
"""GCN encoder Bass kernel builder for trn2 (8 cores, SPMD).

Layout/terminology:
- ncore ranks; each owns lpc real nodes, padded to lpad (%128==0).
- local node l of rank r -> new global id g = q*srcb + r*ql + i, where
  q = l // ql (quarter), i = l % ql, ql = lpad // nq, srcb = ql * ncore.
  Stream s of a source = g // srcb = q. Four AllGathers (one per quarter)
  produce four contiguous table_q tensors in exactly this order.
- dest blocks b = l // 128 (nb per core). Supergroups of sgb blocks share one
  PSUM bank region schedule. Edges stored per (sg, s) run, ordered by block,
  per-(b,s) counts uniform across cores (padded with idx=0/dstv=-1 edges),
  runs padded to multiples of 128.
"""
import numpy as np
import ml_dtypes

import concourse.bacc as bacc
import concourse.mybir as mybir
import concourse.tile as tile
from concourse.library_config import mlp

dt = mybir.dt
BF16 = ml_dtypes.bfloat16


def make_cfg(ncore=8, lpc=12500, lpad=12544, nq=4, sgb=4, h=128):
    nb = lpad // 128
    ql = lpad // nq
    cfg = dict(ncore=ncore, lpc=lpc, lpad=lpad, nq=nq, sgb=sgb, h=h,
               nb=nb, ql=ql, srcb=ql * ncore,
               nsg=(nb + sgb - 1) // sgb)
    assert lpad % 128 == 0 and lpad % nq == 0
    assert cfg["srcb"] <= 32767, "gather idx must fit int16"
    return cfg


def preprocess(cfg, x, edge_index, W1, b1, W2, b2):
    """Returns (meta, in_maps, dinv) for the SPMD program."""
    ncore, lpc, lpad = cfg["ncore"], cfg["lpc"], cfg["lpad"]
    nq, sgb, h = cfg["nq"], cfg["sgb"], cfg["h"]
    nb, ql, srcb, nsg = cfg["nb"], cfg["ql"], cfg["srcb"], cfg["nsg"]
    n = x.shape[0]
    assert n == ncore * lpc

    row = edge_index[0].astype(np.int64)
    col = edge_index[1].astype(np.int64)

    deg = np.bincount(col, minlength=n).astype(np.float64) + 1.0  # + self loop
    dinv = (1.0 / np.sqrt(deg)).astype(np.float32)
    sqdeg = np.sqrt(deg).astype(np.float32)

    # original node id -> (rank, local, new global)
    def to_new_global(node):
        r = node // lpc
        l = node - r * lpc
        q = l // ql
        i = l - q * ql
        return r, l, q * srcb + r * ql + i

    src_r, _, src_g = to_new_global(row)
    dst_r, dst_l, _ = to_new_global(col)
    del src_r

    # --- per-core edge lists (incl self loops), grouped by (sg, s, b) ---
    # count per (core, b, s)
    key_bs = []
    edge_data = []
    for r in range(ncore):
        m = dst_r == r
        e_src = src_g[m]
        e_dl = dst_l[m]
        # self loops for real locals
        ll = np.arange(lpc, dtype=np.int64)
        qq = ll // ql
        sl_g = qq * srcb + r * ql + (ll - qq * ql)
        e_src = np.concatenate([e_src, sl_g])
        e_dl = np.concatenate([e_dl, ll])
        b = e_dl >> 7
        s = e_src // srcb
        key_bs.append(b * nq + s)
        edge_data.append((e_src, e_dl))

    nbs = nb * nq
    cnt_u = np.zeros(nbs, dtype=np.int64)
    for r in range(ncore):
        cnt_u = np.maximum(cnt_u, np.bincount(key_bs[r], minlength=nbs))
    cnt_u = cnt_u.reshape(nb, nq)

    # uniform offsets: runs are per (sg, s): concat of (b in sg, s) segments,
    # each run padded to a multiple of 128.
    seg_off = np.zeros((nb, nq), dtype=np.int64)  # start of (b,s) segment
    run_off = np.zeros((nsg, nq), dtype=np.int64)  # start of (sg,s) run
    run_len = np.zeros((nsg, nq), dtype=np.int64)  # padded length
    pos = 0
    for sg in range(nsg):
        blo, bhi = sg * sgb, min((sg + 1) * sgb, nb)
        for s in range(nq):
            run_off[sg, s] = pos
            for b in range(blo, bhi):
                seg_off[b, s] = pos
                pos += cnt_u[b, s]
            L = pos - run_off[sg, s]
            Lp = -(-L // 128) * 128
            run_len[sg, s] = Lp
            pos = run_off[sg, s] + Lp
    tot = pos  # total padded edge slots (identical all cores)

    # --- fill idx / dst arrays per core ---
    idx_maps = []
    dst_maps = []
    for r in range(ncore):
        e_src, e_dl = edge_data[r]
        kb = key_bs[r]
        order = np.argsort(kb, kind="stable")
        kb_s = kb[order]
        # rank within group
        grp_start = np.searchsorted(kb_s, np.arange(nbs))
        within = np.arange(len(kb_s)) - grp_start[kb_s]
        bs_b, bs_s = kb_s // nq, kb_s % nq
        posn = seg_off[bs_b, bs_s] + within
        ivals = np.zeros(tot, dtype=np.int16)        # pad: gather row 0
        dvals = np.full(tot, -1.0, dtype=np.float32)  # pad: no dest
        ivals[posn] = (e_src[order] - bs_s * srcb).astype(np.int16)
        dvals[posn] = e_dl[order].astype(np.float32)
        idx_maps.append(ivals)
        dst_maps.append(dvals)

    # --- matmul schedule (uniform) ---
    # per (sg, s): list of (chunk_in_run, block); onehot col order = this order
    calls = []      # (sg, s, idx_col_off, n_idx, n_chunk, oh_col_off, n_mm)
    mm_sched = []   # per sg: list of (block, [(s, chunk, oh_col)...])
    oh_cols = 0
    mm_cols_of = {}  # (sg,s) -> list of (chunk, block, col)
    for sg in range(nsg):
        blo, bhi = sg * sgb, min((sg + 1) * sgb, nb)
        for s in range(nq):
            L = run_len[sg, s]
            base = run_off[sg, s]
            cols = []
            col0 = oh_cols
            for b in range(blo, bhi):
                if cnt_u[b, s] == 0:
                    continue
                p0 = seg_off[b, s] - base
                p1 = p0 + cnt_u[b, s]
                for c in range(p0 // 128, (p1 - 1) // 128 + 1):
                    cols.append((c, b, oh_cols))
                    oh_cols += 1
            mm_cols_of[(sg, s)] = cols
            calls.append(dict(sg=sg, s=s, idx_off=base // 16, n_idx=int(L),
                              n_chunk=int(L) // 128, oh_off=col0,
                              n_mm=len(cols)))
        # per-block consecutive matmul order
        blocks = []
        for b in range(blo, bhi):
            lst = []
            for s in range(nq):
                for (c, bb, col) in mm_cols_of[(sg, s)]:
                    if bb == b:
                        lst.append((s, c, col))
            blocks.append((b, lst))
        mm_sched.append(blocks)

    # --- dstv array [128, oh_cols] ---
    dstv_maps = []
    for r in range(ncore):
        dv = dst_maps[r]
        arr = np.full((128, oh_cols), -1.0, dtype=np.float32)
        for sg in range(nsg):
            for s in range(nq):
                base = run_off[sg, s]
                for (c, b, col) in mm_cols_of[(sg, s)]:
                    seg = dv[base + c * 128: base + (c + 1) * 128]
                    local = seg - b * 128
                    ok = (seg >= b * 128) & (seg < (b + 1) * 128)
                    arr[:, col] = np.where(ok, local, -1.0)
        dstv_maps.append(arr.astype(BF16))

    # --- per-core input tensors ---
    iota_tile = np.broadcast_to(
        np.arange(128, dtype=np.float32).astype(BF16)[None, :], (128, 128)).copy()
    in_maps = []
    for r in range(ncore):
        lids = np.arange(lpad)
        real = lids < lpc
        glob = r * lpc + np.minimum(lids, lpc - 1)
        dinv_l = np.where(real, dinv[glob], 0.0).astype(np.float32)
        sqd_l = np.where(real, sqdeg[glob], 0.0).astype(np.float32)
        xs = np.zeros((lpad, h), dtype=np.float32)
        xs[:lpc] = x[r * lpc:(r + 1) * lpc] * dinv_l[:lpc, None]
        xT = np.ascontiguousarray(xs.T).astype(BF16)

        ivals = idx_maps[r]
        icols = tot // 16
        idx_w = ivals.reshape(icols, 16).T  # [16, icols]
        idx_rep = np.ascontiguousarray(np.tile(idx_w, (8, 1)))

        in_maps.append({
            "xT": xT,
            "w1": W1.astype(BF16), "w2": W2.astype(BF16),
            "brow1": b1.astype(BF16)[None, :], "brow2": b2.astype(BF16)[None, :],
            "sqd": sqd_l.astype(BF16)[None, :],
            "dinvw": np.ascontiguousarray(dinv_l.reshape(nb, 128).T),
            "iota": iota_tile,
            "dstv": dstv_maps[r],
            "gidx": idx_rep,
        })

    meta = dict(cfg=cfg, calls=calls, mm_sched=mm_sched, oh_cols=oh_cols,
                tot=tot, run_len=run_len.tolist(), run_off=run_off.tolist(),
                max_chunks=int(run_len.max()) // 128,
                max_mm=max(c["n_mm"] for c in calls))
    return meta, in_maps, dinv


def build_program(meta, msg_bufs=24, oh_bufs=6, psum_bufs=2, debug_stop=None, nlayers=2):
    cfg = meta["cfg"]
    ncore, lpad, nq, h = cfg["ncore"], cfg["lpad"], cfg["nq"], cfg["h"]
    nb, ql, srcb, nsg, sgb = cfg["nb"], cfg["ql"], cfg["srcb"], cfg["nsg"], cfg["sgb"]
    tot, oh_cols = meta["tot"], meta["oh_cols"]
    calls, mm_sched = meta["calls"], meta["mm_sched"]

    nc = bacc.Bacc("TRN2", target_bir_lowering=False, debug=False,
                   num_devices=ncore)

    # I/O
    xT_d = nc.dram_tensor("xT", [h, lpad], dt.bfloat16, kind="ExternalInput")
    w_d = [nc.dram_tensor(f"w{i+1}", [h, h], dt.bfloat16, kind="ExternalInput")
           for i in range(2)]
    brow_d = [nc.dram_tensor(f"brow{i+1}", [1, h], dt.bfloat16, kind="ExternalInput")
              for i in range(2)]
    sqd_d = nc.dram_tensor("sqd", [1, lpad], dt.bfloat16, kind="ExternalInput")
    dinvw_d = nc.dram_tensor("dinvw", [128, nb], dt.float32, kind="ExternalInput")
    iota_d = nc.dram_tensor("iota", [128, 128], dt.bfloat16, kind="ExternalInput")
    dstv_d = nc.dram_tensor("dstv", [128, oh_cols], dt.bfloat16, kind="ExternalInput")
    gidx_d = nc.dram_tensor("gidx", [128, tot // 16], dt.int16, kind="ExternalInput")
    out_d = nc.dram_tensor("out", [lpad, 2 * h], dt.float32, kind="ExternalOutput")

    # internal dram
    agin_q = [nc.dram_tensor(f"agin{q}", [ql, h], dt.bfloat16) for q in range(nq)]
    table_q = [nc.dram_tensor(f"table{q}", [srcb, h], dt.bfloat16,
                              addr_space="Shared") for q in range(nq)]
    h1s_d = nc.dram_tensor("h1s", [lpad, h], dt.bfloat16)

    with tile.TileContext(nc) as tc:
        with (
            tc.tile_pool(name="const", bufs=1) as constp,
            tc.tile_pool(name="hT", bufs=1) as hTp,
            tc.tile_pool(name="msg", bufs=msg_bufs) as msgp,
            tc.tile_pool(name="oh", bufs=oh_bufs) as ohp,
            tc.tile_pool(name="ev", bufs=4) as evp,
            tc.tile_pool(name="gstg", bufs=3) as gstgp,
            tc.tile_pool(name="psum", bufs=psum_bufs, space="PSUM") as psump,
            tc.tile_pool(name="psg", bufs=2, space="PSUM") as psgp,
        ):
            # ---- constants ----
            w_t = []
            brow_t = []
            for i in range(2):
                wt = constp.tile([h, h], dt.bfloat16, tag=f"w{i}")
                nc.sync.dma_start(wt[:], w_d[i][:])
                w_t.append(wt)
                bt = constp.tile([1, h], dt.bfloat16, tag=f"b{i}")
                nc.sync.dma_start(bt[:], brow_d[i][:])
                brow_t.append(bt)
            sqd_t = constp.tile([1, lpad], dt.bfloat16, tag="sqd")
            nc.sync.dma_start(sqd_t[:], sqd_d[:])
            dinvw_t = constp.tile([128, nb], dt.float32, tag="dinvw")
            nc.sync.dma_start(dinvw_t[:], dinvw_d[:])
            iota_t = constp.tile([128, 128], dt.bfloat16, tag="iota")
            nc.sync.dma_start(iota_t[:], iota_d[:])
            dstv_t = constp.tile([128, oh_cols], dt.bfloat16, tag="dstv")
            nc.sync.dma_start(dstv_t[:], dstv_d[:])
            gidx_t = constp.tile([128, tot // 16], dt.int16, tag="gidx")
            nc.sync.dma_start(gidx_t[:], gidx_d[:])
            xT_t = constp.tile([h, lpad], dt.bfloat16, tag="xT")
            nc.sync.dma_start(xT_t[:], xT_d[:])
            h1sT_t = constp.tile([h, lpad], dt.bfloat16, tag="h1sT")

            def emit_layer(layer, hT_in):
                w = w_t[layer]
                brow = brow_t[layer]
                # ---- GEMM (node-major) + write table slices + AllGather ----
                # process per quarter so AG_q can fire early
                for q in range(nq):
                    r0, r1 = q * ql, (q + 1) * ql
                    b0, b1_ = r0 // 128, (r1 + 127) // 128
                    for gb in range(b0, b1_, 4):
                        gbe = min(gb + 4, b1_)
                        ps = psgp.tile([128, 512], dt.float32, tag="gemm")
                        stg = gstgp.tile([128, 512], dt.bfloat16, tag="gstg")
                        for sub, b in enumerate(range(gb, gbe)):
                            nc.tensor.matmul(
                                ps[:, sub * 128:(sub + 1) * 128],
                                hT_in[:, b * 128:(b + 1) * 128],
                                w[:], start=True, stop=True,
                                skip_group_check=True)
                        nsub = gbe - gb
                        nc.vector.tensor_copy(stg[:, :nsub * 128], ps[:, :nsub * 128])
                        for sub, b in enumerate(range(gb, gbe)):
                            # rows b*128..b*128+128 may straddle quarters
                            lo = b * 128
                            hi = lo + 128
                            cl = max(lo, r0)
                            ch = min(hi, r1)
                            if ch <= cl:
                                continue
                            nc.sync.dma_start(
                                agin_q[q][cl - r0: ch - r0, :],
                                stg[cl - lo: ch - lo, sub * 128:(sub + 1) * 128],
                            )
                    nc.gpsimd.collective_compute(
                        "AllGather", mybir.AluOpType.bypass,
                        replica_groups=[list(range(ncore))],
                        ins=[agin_q[q][:]], outs=[table_q[q][:]])
                if debug_stop == "ag":
                    nc.gpsimd.dma_start(out_d[0:128, layer * h:(layer + 1) * h],
                                        table_q[0][0:128, :])
                    return

                # ---- gather + segsum per supergroup ----
                for sg in range(nsg):
                    msg_ts = {}
                    oh_ts = {}
                    for cinfo in calls:
                        if cinfo["sg"] != sg:
                            continue
                        s = cinfo["s"]
                        nck = cinfo["n_chunk"]
                        if nck == 0:
                            continue
                        subs = []
                        for k0 in range(0, nck, 8):
                            k1 = min(k0 + 8, nck)
                            ni = (k1 - k0) * 128
                            mt = msgp.tile([128, k1 - k0, h], dt.bfloat16,
                                           tag="msg")
                            nc.gpsimd.dma_gather(
                                mt[:], table_q[s][:],
                                gidx_t[:, cinfo["idx_off"] + k0 * 8:
                                       cinfo["idx_off"] + k0 * 8 + ni // 16],
                                ni, ni, h)
                            subs.append(mt)
                        msg_ts[s] = (subs, cinfo)
                        if debug_stop == "gather_noh":
                            continue
                        nmm = cinfo["n_mm"]
                        ot = ohp.tile([128, nmm, 128], dt.bfloat16, tag="oh")
                        iota_b = iota_t[:].unsqueeze(1).broadcast_to([128, nmm, 128])
                        dv_b = (dstv_t[:, cinfo["oh_off"]: cinfo["oh_off"] + nmm]
                                .unsqueeze(2).broadcast_to([128, nmm, 128]))
                        nc.vector.tensor_tensor(ot[:], iota_b, dv_b,
                                                mybir.AluOpType.is_equal)
                        oh_ts[s] = (ot, cinfo)

                    if debug_stop in ("gather", "gather_noh"):
                        if sg == 0:
                            mt0 = msg_ts[0][0][0]
                            nc.gpsimd.dma_start(
                                out_d[0:128, layer * h:(layer + 1) * h],
                                mt0[:, 0, :])
                        continue
                    blocks = mm_sched[sg]
                    blo = sg * sgb
                    ps = psump.tile([128, 512], dt.float32, tag="seg")
                    for (b, lst) in blocks:
                        sub = b - blo
                        reg = ps[:, sub * 128:(sub + 1) * 128]
                        nc.tensor.matmul(
                            reg, sqd_t[0:1, b * 128:(b + 1) * 128], brow[:],
                            start=True, stop=False, skip_group_check=True)
                        for j, (s, c, col) in enumerate(lst):
                            subs, cinfo = msg_ts[s]
                            ot, oinfo = oh_ts[s]
                            nc.tensor.matmul(
                                reg, ot[:, col - oinfo["oh_off"], :],
                                subs[c // 8][:, c % 8, :],
                                start=False, stop=(j == len(lst) - 1),
                                skip_group_check=True)
                    # evict
                    for (b, lst) in blocks:
                        sub = b - blo
                        reg = ps[:, sub * 128:(sub + 1) * 128]
                        of = evp.tile([128, 128], dt.float32, tag="outf")
                        nc.scalar.activation(
                            of[:], reg, mybir.ActivationFunctionType.Relu,
                            bias=0.0, scale=dinvw_t[:, b:b + 1])
                        nc.sync.dma_start(
                            out_d[b * 128:(b + 1) * 128,
                                  layer * h:(layer + 1) * h], of[:])
                        if layer == 0:
                            hs = evp.tile([128, 128], dt.bfloat16, tag="h1s")
                            nc.vector.tensor_scalar(
                                hs[:], of[:], dinvw_t[:, b:b + 1], None,
                                mybir.AluOpType.mult)
                            nc.sync.dma_start(
                                h1s_d[b * 128:(b + 1) * 128, :], hs[:])

            emit_layer(0, xT_t)
            if nlayers > 1:
                # transpose-read h1s into feature-major
                for q in range(nq):
                    nc.sync.dma_start(
                        h1sT_t[:, q * ql:(q + 1) * ql],
                        h1s_d[q * ql:(q + 1) * ql, :], transpose=True)
                emit_layer(1, h1sT_t)

    nc.compile()
    return nc


def run_full(cfg, x, edge_index, W1, b1, W2, b2, **bkw):
    from concourse.bass_utils import run_bass_kernel_spmd
    meta, in_maps, dinv = preprocess(cfg, x, edge_index, W1, b1, W2, b2)
    nc = build_program(meta, **bkw)
    res = run_bass_kernel_spmd(nc, in_maps, list(range(cfg["ncore"])))
    lpc, lpad, h = cfg["lpc"], cfg["lpad"], cfg["h"]
    out = np.concatenate(
        [res.results[r]["out"][:lpc] for r in range(cfg["ncore"])], axis=0)
    return out, res


# ---------------------------------------------------------------------------
# Full-size problem entry point
# ---------------------------------------------------------------------------
N, E, DIN, H = 100000, 1600000, 128, 128
N_CORES = 8

_CACHE = {}


def _prepare(x, edge_index, W1, b1, W2, b2, **bkw):
    cfg = make_cfg(ncore=N_CORES, lpc=N // N_CORES, lpad=12544, nq=4, sgb=4)
    meta, in_maps, _ = preprocess(cfg, np.asarray(x, np.float32),
                                  np.asarray(edge_index), np.asarray(W1),
                                  np.asarray(b1), np.asarray(W2),
                                  np.asarray(b2))
    nc = build_program(meta, **bkw)
    return cfg, meta, in_maps, nc


def _run(x, edge_index, W1, b1, W2, b2, **bkw):
    from concourse.bass_utils import run_bass_kernel_spmd
    cfg, meta, in_maps, nc = _prepare(x, edge_index, W1, b1, W2, b2, **bkw)
    res = run_bass_kernel_spmd(nc, in_maps, list(range(N_CORES)))
    lpc = cfg["lpc"]
    out = np.concatenate(
        [res.results[r]["out"][:lpc] for r in range(N_CORES)], axis=0)
    return out.astype(np.float32), (cfg, meta, in_maps, nc, res)


def kernel(x, edge_index, W1, b1, W2, b2):
    out, _ = _run(x, edge_index, W1, b1, W2, b2)
    return out
